# revision 1
# baseline (speedup 1.0000x reference)
"""Trainium2 Bass kernel for a 2-layer GCN + global mean pool + MLP head.

Strategy (8 NeuronCores, SPMD, one shared NEFF):
  - Nodes (= aggregation dsts) are sharded across cores: core c owns rows
    [c*12500, (c+1)*12500), padded to 12800 = 25 blocks of 512.
  - Layer 1 is computed as agg = A_norm @ x (gather + one-hot-matmul
    segment-sum on device), then h1e = elu(agg.T @ W1 + b1) per shard.
  - Layer 2 + global mean pool collapse algebraically: the pooled sums are
    (B^T A_norm) h1e W2 with B the graph one-hot; wmat = B^T A_norm is a
    [64 x N] matrix computed on host from edge_index alone (graph
    partitioning / preprocessing, per the sharding hint). Each core
    computes partial = wmat[:, shard].T-contraction with h1e on device.
  - Host epilogue: sum 8 partials, @W2, mean, fc1/relu/fc2, log_softmax
    (~0.003% of FLOPs).

  Edges (incl. self loops) are routed on host to fixed "slot tiles":
  run = (dst block of 512, src chunk of 25000), T_RUN=23 tiles of 128 edge
  slots each; tile j accumulates into PSUM columns [WBASE[j], WBASE[j]+48)
  of its block. The gathered 128-edge tile is the matmul stationary
  operand; the moving operand is a 48-wide one-hot built on the vector
  engine from per-edge window-relative dst ids (is_equal vs iota) scaled
  by the GCN norm. All instruction structure (incl. PSUM offsets) is
  input-independent, so the NEFF is reusable for any same-shape input.

  Overflow valve: edges that don't fit their run's slots (zero for random
  graphs by construction) are pre-aggregated on host into aggX, which also
  serves as the PSUM initializer via an identity-matmul.
"""
import sys
import numpy as np

sys.path.insert(0, "/opt/trn_rl_repo")


# ---------------------------------------------------------------- config ----
class CFG:
    def __init__(self, N=100000, D=128, G=64, n_cores=8, n_chunk=4, blk=512,
                 t_run=23, w=48, stride=24):
        self.N, self.D, self.G = N, D, G
        self.N_CORES, self.N_CHUNK, self.BLK = n_cores, n_chunk, blk
        self.T_RUN, self.W, self.STRIDE = t_run, w, stride
        assert N % n_cores == 0 and N % n_chunk == 0
        self.SHARD = N // n_cores
        self.CHUNK = N // n_chunk
        assert self.CHUNK <= 32768, "src chunk ids must fit int16"
        self.N_BLK = (self.SHARD + blk - 1) // blk
        self.SHARD_PAD = self.N_BLK * blk
        self.WBASE = np.clip((np.arange(t_run) - 1) * stride, 0, blk - w)
        self.SLOTS = t_run * 128
        self.N_RUN = self.N_BLK * n_chunk
        self.NT = blk // 128  # node tiles per block


FULL = CFG()


# ---------------------------------------------------------- preprocessing ----
def preprocess(cfg, x, edge_index, batch):
    """Host-side graph partitioning: per-core gather/one-hot streams."""
    src = np.asarray(edge_index[0], dtype=np.int64)
    dst = np.asarray(edge_index[1], dtype=np.int64)
    batch = np.asarray(batch, dtype=np.int64)
    N, G = cfg.N, cfg.G

    deg = np.bincount(dst, minlength=N).astype(np.float32) + 1.0  # + self loop
    dinv = (1.0 / np.sqrt(deg)).astype(np.float32)

    loops = np.arange(N, dtype=np.int64)
    src_all = np.concatenate([src, loops])
    dst_all = np.concatenate([dst, loops])
    norm_all = (dinv[src_all] * dinv[dst_all]).astype(np.float32)

    flat = batch[dst_all] * N + src_all
    wmat = np.bincount(flat, weights=norm_all.astype(np.float64),
                       minlength=G * N).reshape(G, N).astype(np.float32)

    wb, W, T_RUN, BLK = cfg.WBASE, cfg.W, cfg.T_RUN, cfg.BLK
    per_core = []
    n_overflow = 0
    for c in range(cfg.N_CORES):
        m = (dst_all >= c * cfg.SHARD) & (dst_all < (c + 1) * cfg.SHARD)
        s_c = src_all[m]
        dl_c = dst_all[m] - c * cfg.SHARD
        nv_c = norm_all[m]
        blk = dl_c // BLK
        chunk = s_c // cfg.CHUNK
        o = np.lexsort((dl_c, chunk, blk))
        s_c, dl_c, nv_c, blk, chunk = s_c[o], dl_c[o], nv_c[o], blk[o], chunk[o]

        gidx = np.zeros((cfg.N_RUN, cfg.SLOTS), dtype=np.int16)
        dstrel = np.full((cfg.N_RUN, cfg.SLOTS), -1.0, dtype=np.float32)
        normv = np.zeros((cfg.N_RUN, cfg.SLOTS), dtype=np.float32)
        aggX = np.zeros((cfg.SHARD_PAD, cfg.D), dtype=np.float32)

        run_id = blk * cfg.N_CHUNK + chunk
        run_starts = np.searchsorted(run_id, np.arange(cfg.N_RUN))
        run_ends = np.searchsorted(run_id, np.arange(cfg.N_RUN) + 1)
        for r in range(cfg.N_RUN):
            a, b = int(run_starts[r]), int(run_ends[r])
            if a == b:
                continue
            bb = (r // cfg.N_CHUNK) * BLK
            ch = r % cfg.N_CHUNK
            drel = dl_c[a:b] - bb
            # exact greedy: leftmost eligible tile, ascending dst
            starts = np.searchsorted(drel, wb)
            ends = np.searchsorted(drel, wb + W)
            tile_of = np.full(b - a, -1, dtype=np.int64)
            placed = 0
            for j in range(T_RUN):
                lo = max(placed, int(starts[j]))
                take = min(128, int(ends[j]) - lo)
                if take > 0:
                    tile_of[lo:lo + take] = j
                    placed = lo + take
                else:
                    placed = max(placed, lo)
            okm = tile_of >= 0
            tloc = tile_of[okm]
            loads = np.bincount(tloc, minlength=T_RUN)
            cum = np.concatenate([[0], np.cumsum(loads)])[:-1]
            pos = np.arange(len(tloc)) - np.repeat(cum, loads)
            slot = tloc * 128 + pos
            gidx[r, slot] = (s_c[a:b][okm] - ch * cfg.CHUNK).astype(np.int16)
            dstrel[r, slot] = (drel[okm] - wb[tloc]).astype(np.float32)
            normv[r, slot] = nv_c[a:b][okm]
            ovf = np.nonzero(~okm)[0]
            if len(ovf):
                n_overflow += len(ovf)
                for i in ovf:
                    aggX[dl_c[a + i]] += nv_c[a + i] * np.asarray(
                        x[s_c[a + i]], dtype=np.float32)

        wT = np.zeros((cfg.SHARD_PAD, G), dtype=np.float32)
        wT[:cfg.SHARD] = wmat[:, c * cfg.SHARD:(c + 1) * cfg.SHARD].T

        per_core.append({
            # wrapped layouts for the device
            # wrapped [16, SLOTS//16] then replicated to 128 partitions
            # (each of the 8 gpsimd sub-cores reads its own 16 partitions)
            "gidx": np.ascontiguousarray(np.tile(
                gidx.reshape(cfg.N_RUN, cfg.SLOTS // 16, 16)
                    .transpose(0, 2, 1), (1, 8, 1))),
            "dstrel": np.ascontiguousarray(
                dstrel.reshape(cfg.N_RUN, T_RUN, 128).transpose(0, 2, 1)
            ).astype(np.float16),
            "normv": np.ascontiguousarray(
                normv.reshape(cfg.N_RUN, T_RUN, 128).transpose(0, 2, 1)
            ).astype(np.float16),
            "aggX": np.ascontiguousarray(aggX.T),      # [D, SHARD_PAD]
            "wT": wT,                                   # [SHARD_PAD, G]
        })

    cnt = np.bincount(batch, minlength=G).astype(np.float32)
    return per_core, cnt, n_overflow


def pack_x(x):
    """[N, D] f32 -> [N, 2D] fp16: per row [hi | lo] with hi+lo == x exactly
    to fp16^2 precision."""
    xf = np.asarray(x, dtype=np.float32)
    hi = xf.astype(np.float16)
    lo = (xf - hi.astype(np.float32)).astype(np.float16)
    return np.ascontiguousarray(np.concatenate([hi, lo], axis=1))


# ---------------------------------------------------------- bass kernel ----
def build_kernel(cfg):
    from concourse import bacc, bass, tile, mybir
    from concourse.masks import make_identity
    f32 = mybir.dt.float32
    f16 = mybir.dt.float16
    bf16 = mybir.dt.bfloat16

    nc = bacc.Bacc("TRN2", target_bir_lowering=False, debug=False,
                   enable_asserts=False)
    x_t = nc.dram_tensor("x", [cfg.N, 2 * cfg.D], f16, kind="ExternalInput")
    gidx_t = nc.dram_tensor("gidx", [cfg.N_RUN, 128, cfg.SLOTS // 16],
                            mybir.dt.int16, kind="ExternalInput")
    dstrel_t = nc.dram_tensor("dstrel", [cfg.N_RUN, 128, cfg.T_RUN], f16,
                              kind="ExternalInput")
    normv_t = nc.dram_tensor("normv", [cfg.N_RUN, 128, cfg.T_RUN], f16,
                             kind="ExternalInput")
    aggx_t = nc.dram_tensor("aggX", [cfg.D, cfg.SHARD_PAD], bf16,
                            kind="ExternalInput")
    wT_t = nc.dram_tensor("wT", [cfg.SHARD_PAD, cfg.G], f32,
                          kind="ExternalInput")
    W1_t = nc.dram_tensor("W1", [cfg.D, cfg.D], f32, kind="ExternalInput")
    b1_t = nc.dram_tensor("b1", [1, cfg.D], f32, kind="ExternalInput")
    out_t = nc.dram_tensor("partial", [cfg.G, cfg.D], f32,
                           kind="ExternalOutput")

    T, W, NT, BLK = cfg.T_RUN, cfg.W, cfg.NT, cfg.BLK
    WB = [int(v) for v in cfg.WBASE]
    eq, mult, add, amax = (mybir.AluOpType.is_equal, mybir.AluOpType.mult,
                           mybir.AluOpType.add, mybir.AluOpType.max)

    with tile.TileContext(nc) as tc:
        with (tc.tile_pool(name="const", bufs=1) as cpool,
              tc.tile_pool(name="gbuf", bufs=3) as gpool,
              tc.tile_pool(name="meta", bufs=8) as mpool,
              tc.tile_pool(name="oh", bufs=3) as opool,
              tc.tile_pool(name="agg", bufs=2) as apool,
              tc.tile_pool(name="eluv", bufs=2) as epool,
              tc.tile_pool(name="wt", bufs=2) as wpool,
              tc.tile_pool(name="ax", bufs=2) as axpool,
              tc.tile_pool(name="psA", bufs=2, space="PSUM") as pApool,
              tc.tile_pool(name="psB", bufs=2, space="PSUM") as pBpool,
              tc.tile_pool(name="psC", bufs=1, space="PSUM") as pCpool,
              tc.tile_pool(name="outp", bufs=1) as outpool):

            ident = cpool.tile([128, 128], bf16)
            make_identity(nc, ident[:])
            W1s = cpool.tile([128, cfg.D], f32)
            nc.sync.dma_start(W1s[:], W1_t.ap())
            b1s = cpool.tile([128, cfg.D], f32)
            nc.sync.dma_start(b1s[:], b1_t.ap().to_broadcast((128, cfg.D)))
            iota = cpool.tile([128, W], f16)
            nc.gpsimd.iota(iota[:], pattern=[[1, W]], base=0,
                           channel_multiplier=0,
                           allow_small_or_imprecise_dtypes=True)

            psC = pCpool.tile([cfg.G, cfg.D], f32)

            for b in range(cfg.N_BLK):
                axs = axpool.tile([128, BLK], bf16)
                nc.sync.dma_start(axs[:], aggx_t.ap()[:, b * BLK:(b + 1) * BLK])
                psA = pApool.tile([128, BLK], f32)
                nc.tensor.matmul(out=psA[:], lhsT=ident[:], rhs=axs[:],
                                 start=True, stop=False)
                for ch in range(cfg.N_CHUNK):
                    r = b * cfg.N_CHUNK + ch
                    gxs = mpool.tile([128, cfg.SLOTS // 16], mybir.dt.int16,
                                     tag="gx")
                    nc.sync.dma_start(gxs[:], gidx_t.ap()[r])
                    drs = mpool.tile([128, T], f16, tag="dr")
                    nc.sync.dma_start(drs[:], dstrel_t.ap()[r])
                    nvs = mpool.tile([128, T], f16, tag="nv")
                    nc.sync.dma_start(nvs[:], normv_t.ap()[r])

                    gb = gpool.tile([128, T, 2 * cfg.D], f16)
                    nc.gpsimd.dma_gather(
                        out_ap=gb[:],
                        in_ap=x_t.ap()[ch * cfg.CHUNK:(ch + 1) * cfg.CHUNK, :],
                        idxs_ap=gxs[:],
                        num_idxs=cfg.SLOTS,
                        num_idxs_reg=cfg.SLOTS,
                        elem_size=2 * cfg.D,
                        single_packet=False,
                    )

                    oh = opool.tile([128, T, W], f16)
                    nc.vector.tensor_tensor(
                        out=oh[:],
                        in0=drs[:].unsqueeze(2).broadcast_to((128, T, W)),
                        in1=iota[:].unsqueeze(1).broadcast_to((128, T, W)),
                        op=eq)
                    nc.vector.tensor_tensor(
                        out=oh[:], in0=oh[:],
                        in1=nvs[:].unsqueeze(2).broadcast_to((128, T, W)),
                        op=mult)

                    for t in range(T):
                        last = (ch == cfg.N_CHUNK - 1 and t == T - 1)
                        nc.tensor.matmul(
                            out=psA[:, WB[t]:WB[t] + W],
                            lhsT=gb[:, t, 0:cfg.D],
                            rhs=oh[:, t, :],
                            start=False, stop=False,
                        )
                        nc.tensor.matmul(
                            out=psA[:, WB[t]:WB[t] + W],
                            lhsT=gb[:, t, cfg.D:2 * cfg.D],
                            rhs=oh[:, t, :],
                            start=False, stop=last,
                        )

                # drain agg (feat-major [D x BLK])
                aggs = apool.tile([128, BLK], f32)
                nc.vector.tensor_copy(out=aggs[:], in_=psA[:])

                # B: h1 = agg.T @ W1  -> psB [node x feat_out], per node tile
                psB = pBpool.tile([128, BLK], f32)
                for nt in range(NT):
                    nc.tensor.matmul(out=psB[:, nt * cfg.D:(nt + 1) * cfg.D],
                                     lhsT=aggs[:, nt * 128:(nt + 1) * 128],
                                     rhs=W1s[:], start=True, stop=True)

                # elu(x+b1) = max(xb, min(exp(xb)-1, 0))
                xb = epool.tile([128, NT, cfg.D], f32, tag="xb")
                nc.vector.tensor_tensor(
                    out=xb[:],
                    in0=psB[:].rearrange("p (t d) -> p t d", d=cfg.D),
                    in1=b1s[:].unsqueeze(1).broadcast_to((128, NT, cfg.D)),
                    op=add)
                ex = epool.tile([128, NT * cfg.D], f32, tag="ex")
                nc.scalar.activation(
                    out=ex[:], in_=xb[:].rearrange("p t d -> p (t d)"),
                    func=mybir.ActivationFunctionType.Exp)
                nc.vector.tensor_scalar(
                    out=ex[:], in0=ex[:], scalar1=-1.0, scalar2=0.0,
                    op0=add, op1=mybir.AluOpType.min)
                h1e = epool.tile([128, NT * cfg.D], f32, tag="h1e")
                nc.vector.tensor_tensor(
                    out=h1e[:], in0=xb[:].rearrange("p t d -> p (t d)"),
                    in1=ex[:], op=amax)

                # C: partial += wT_block.T @ h1e
                wts = wpool.tile([128, NT, cfg.G], f32)
                nc.sync.dma_start(
                    wts[:],
                    wT_t.ap()[b * BLK:(b + 1) * BLK, :]
                        .rearrange("(t p) g -> p t g", p=128))
                for nt in range(NT):
                    nc.tensor.matmul(
                        out=psC[:],
                        lhsT=wts[:, nt, :],
                        rhs=h1e[:, nt * cfg.D:(nt + 1) * cfg.D],
                        start=(b == 0 and nt == 0),
                        stop=(b == cfg.N_BLK - 1 and nt == NT - 1),
                    )

            outs = outpool.tile([cfg.G, cfg.D], f32)
            nc.vector.tensor_copy(out=outs[:], in_=psC[:])
            nc.sync.dma_start(out_t.ap(), outs[:])

    nc.compile()
    return nc


# ------------------------------------------------------------- epilogue ----
def epilogue(partials, cnt, W2, b2, fc1_W, fc1_b, fc2_W, fc2_b):
    g_sum = np.sum(partials, axis=0, dtype=np.float32)
    S = g_sum @ W2 + cnt[:, None] * b2[None, :]
    mean = S / np.maximum(cnt, 1.0)[:, None]
    z = np.maximum(mean @ fc1_W + fc1_b[None, :], 0.0)
    z = z @ fc2_W + fc2_b[None, :]
    zmax = z.max(axis=1, keepdims=True)
    lse = np.log(np.sum(np.exp(z - zmax), axis=1, keepdims=True)) + zmax
    return (z - lse).astype(np.float32)


_NC_CACHE = {}


def run_on_device(cfg, per_core, x, trace=False):
    from concourse import bass_utils
    key = (cfg.N, cfg.D, cfg.G, cfg.N_CORES)
    if key not in _NC_CACHE:
        _NC_CACHE[key] = build_kernel(cfg)
    nc = _NC_CACHE[key]
    import ml_dtypes
    xp = pack_x(x)
    b1z = np.zeros((1, cfg.D), dtype=np.float32)
    in_maps = []
    for c in range(cfg.N_CORES):
        s = per_core[c]
        in_maps.append({
            "x": xp, "gidx": s["gidx"], "dstrel": s["dstrel"],
            "normv": s["normv"],
            "aggX": s["aggX"].astype(ml_dtypes.bfloat16), "wT": s["wT"],
            "W1": None, "b1": b1z,
        })
    return nc, in_maps


def kernel(x, edge_index, batch, W1, b1, W2, b2, fc1_W, fc1_b, fc2_W, fc2_b):
    from concourse import bass_utils
    cfg = FULL
    per_core, cnt, _ = preprocess(cfg, x, edge_index, batch)
    nc, in_maps = run_on_device(cfg, per_core, x)
    W1f = np.ascontiguousarray(np.asarray(W1, dtype=np.float32))
    b1f = np.asarray(b1, dtype=np.float32).reshape(1, cfg.D)
    for m in in_maps:
        m["W1"] = W1f
        m["b1"] = b1f
    res = bass_utils.run_bass_kernel_spmd(
        nc, in_maps, core_ids=list(range(cfg.N_CORES)))
    partials = [res.results[c]["partial"] for c in range(cfg.N_CORES)]
    out = epilogue(partials, cnt,
                   np.asarray(W2, np.float32), np.asarray(b2, np.float32),
                   np.asarray(fc1_W, np.float32), np.asarray(fc1_b, np.float32),
                   np.asarray(fc2_W, np.float32), np.asarray(fc2_b, np.float32))
    return out



# revision 2
# speedup vs baseline: 1.0128x; 1.0128x over previous
"""Trainium2 Bass kernel for a 2-layer GCN + global mean pool + MLP head.

Device work per core (SPMD, shared NEFF):
  - agg = A_norm @ x for the core's 12.5K-node shard: per-edge dma_gather of
    fp16 x rows (256B elems) + one-hot window matmuls into PSUM.
    * 50 gathers (25 blocks x 2 src chunks), round-robined over 4 SWDGE
      queues so descriptor generation uses all 8 Q7 cores.
    * src chunks of 50K rows via signed int16 indices against a centered
      base (the gather ucode sign-extends and MULUS-accumulates).
    * one-hot tiles precomputed on host (graph metadata only) and DMA'd,
      keeping the vector engine almost idle (it contends with SWDGE).
  - h1e = elu(agg.T @ W1 + b1); elementwise on the scalar engine.
  - partial[G, D] += wT_block.T @ h1e with wT = (B^T A_norm).T from host.
  - Host epilogue: sum partials, @W2, mean, fc1/relu/fc2, log_softmax.
"""
import sys
import numpy as np

sys.path.insert(0, "/opt/trn_rl_repo")


# ---------------------------------------------------------------- config ----
class CFG:
    def __init__(self, N=100000, D=128, G=64, n_cores=8, n_chunk=2, blk=512,
                 w=32):
        self.N, self.D, self.G = N, D, G
        self.N_CORES, self.N_CHUNK, self.BLK, self.W = n_cores, n_chunk, blk, w
        self.SHARD = N // n_cores
        self.CHUNK = N // n_chunk
        self.HALF = self.CHUNK // 2  # centered gather base offset
        assert self.HALF <= 32768
        self.N_BLK = (self.SHARD + blk - 1) // blk
        self.SHARD_PAD = self.N_BLK * blk
        self.N_RUN = self.N_BLK * n_chunk
        self.NT = blk // 128


FULL = CFG()


# ---------------------------------------------------------- preprocessing ----
def pack_run_joint(drels, W, blk):
    """Joint greedy window packing across cores: one shared base list; each
    core fills <=128 of its own (sorted) edges per tile."""
    nc_ = len(drels)
    ptrs = [0] * nc_
    tile_ofs = [np.full(len(d), -1, dtype=np.int64) for d in drels]
    wbases = []
    t = 0
    while True:
        lo = blk
        for c in range(nc_):
            if ptrs[c] < len(drels[c]):
                lo = min(lo, int(drels[c][ptrs[c]]))
        if lo >= blk:
            break
        base = min(lo, blk - W)
        for c in range(nc_):
            a = ptrs[c]
            if a >= len(drels[c]):
                continue
            hi = np.searchsorted(drels[c], base + W, side="left")
            j = min(a + 128, hi)
            tile_ofs[c][a:j] = t
            ptrs[c] = j
        wbases.append(base)
        t += 1
    return tile_ofs, wbases


def preprocess(cfg, edge_index, batch):
    src = np.asarray(edge_index[0], dtype=np.int64)
    dst = np.asarray(edge_index[1], dtype=np.int64)
    batch = np.asarray(batch, dtype=np.int64)
    N, G, W, BLK = cfg.N, cfg.G, cfg.W, cfg.BLK

    deg = np.bincount(dst, minlength=N).astype(np.float32) + 1.0
    dinv = (1.0 / np.sqrt(deg)).astype(np.float32)

    loops = np.arange(N, dtype=np.int64)
    la = np.concatenate([src, loops])
    lb = np.concatenate([dst, loops])
    wnorm = (dinv[la] * dinv[lb]).astype(np.float32)
    flat = batch[lb] * N + la
    wmat = np.bincount(flat, weights=wnorm.astype(np.float64),
                       minlength=G * N).reshape(G, N).astype(np.float32)

    # self-loops are folded into the PSUM init (dinv^2-scaled x block rows
    # via DMA-transpose); only real edges go through the gather stream.
    src_all, dst_all = src, dst
    norm_all = (dinv[src_all] * dinv[dst_all]).astype(np.float32)

    cores = []
    for c in range(cfg.N_CORES):
        m = (dst_all >= c * cfg.SHARD) & (dst_all < (c + 1) * cfg.SHARD)
        s_c = src_all[m]
        dl_c = dst_all[m] - c * cfg.SHARD
        nv_c = norm_all[m]
        blk_id = dl_c // BLK
        chunk = s_c // cfg.CHUNK
        o = np.lexsort((dl_c, chunk, blk_id))
        s_c, dl_c, nv_c = s_c[o], dl_c[o], nv_c[o]
        run_id = blk_id[o] * cfg.N_CHUNK + chunk[o]
        run_starts = np.searchsorted(run_id, np.arange(cfg.N_RUN))
        run_ends = np.searchsorted(run_id, np.arange(cfg.N_RUN) + 1)
        cores.append((s_c, dl_c, nv_c, run_starts, run_ends))

    run_T, run_wb = [], []
    core_gidx = [[] for _ in range(cfg.N_CORES)]
    core_oh = [[] for _ in range(cfg.N_CORES)]
    for r in range(cfg.N_RUN):
        bb = (r // cfg.N_CHUNK) * BLK
        ch = r % cfg.N_CHUNK
        drels = []
        for c in range(cfg.N_CORES):
            s_c, dl_c, nv_c, rs, re = cores[c]
            a, b = int(rs[r]), int(re[r])
            drels.append(dl_c[a:b] - bb)
        tile_ofs, wbases = pack_run_joint(drels, W, BLK)
        T_r = max(len(wbases), 1)
        wbases = wbases or [0]
        S_r = T_r * 128
        wb_arr = np.asarray(wbases, dtype=np.int64)
        for c in range(cfg.N_CORES):
            s_c, dl_c, nv_c, rs, re = cores[c]
            a, b = int(rs[r]), int(re[r])
            tile_of = tile_ofs[c]
            assert b == a or (tile_of >= 0).all()
            gidx = np.zeros(S_r, dtype=np.int16)
            oh = np.zeros((T_r, 128, W), dtype=np.float16)
            if b > a:
                loads = np.bincount(tile_of, minlength=T_r)
                cum = np.concatenate([[0], np.cumsum(loads)])[:-1]
                pos = np.arange(b - a) - np.repeat(cum, loads)
                slot = tile_of * 128 + pos
                gidx[slot] = (s_c[a:b] - ch * cfg.CHUNK
                              - cfg.HALF).astype(np.int16)
                wrel = (dl_c[a:b] - bb - wb_arr[tile_of]).astype(np.int64)
                oh[tile_of, pos, wrel] = nv_c[a:b].astype(np.float16)
                # trailing-negative sentinel guard: last slot idx must be >= 0
                if gidx[S_r - 1] < 0:
                    lt = T_r - 1
                    cand = np.nonzero(gidx[lt * 128:] >= 0)[0]
                    assert len(cand) > 0, "all-negative last tile"
                    sw = lt * 128 + int(cand[0])
                    for arr in (gidx,):
                        arr[sw], arr[S_r - 1] = arr[S_r - 1], arr[sw]
                    tmp = oh[lt, cand[0]].copy()
                    oh[lt, cand[0]] = oh[lt, 127]
                    oh[lt, 127] = tmp
            gw = np.tile(gidx.reshape(S_r // 16, 16).T, (8, 1))
            core_gidx[c].append(gw)
            core_oh[c].append(oh.transpose(1, 0, 2).reshape(128, T_r * W))
        run_T.append(T_r)
        run_wb.append(wbases)

    import ml_dtypes
    per_core = []
    for c in range(cfg.N_CORES):
        wT = np.zeros((cfg.SHARD_PAD, G), dtype=ml_dtypes.bfloat16)
        wT[:cfg.SHARD] = wmat[:, c * cfg.SHARD:(c + 1) * cfg.SHARD].T
        per_core.append({
            "gidx": np.ascontiguousarray(np.concatenate(core_gidx[c], axis=1)),
            "oh": np.ascontiguousarray(np.concatenate(core_oh[c], axis=1)),
            "wT": wT,
            "dinv2": (dinv[c * cfg.SHARD:(c + 1) * cfg.SHARD] ** 2
                      ).astype(np.float32),
        })

    cnt = np.bincount(batch, minlength=G).astype(np.float32)
    return per_core, cnt, run_T, run_wb


# ---------------------------------------------------------- bass kernel ----
def build_kernel(cfg, run_T, run_wb):
    from concourse import bacc, bass, tile, mybir
    from concourse.masks import make_identity
    f32 = mybir.dt.float32
    f16 = mybir.dt.float16
    bf16 = mybir.dt.bfloat16

    T_total = sum(run_T)
    nc = bacc.Bacc("TRN2", target_bir_lowering=False, debug=False,
                   enable_asserts=False, num_swdge_queues=4)
    x_t = nc.dram_tensor("x", [cfg.N, cfg.D], f16, kind="ExternalInput")
    xself_t = nc.dram_tensor("x_selfT", [cfg.D, cfg.SHARD_PAD], f16,
                             kind="ExternalInput")
    gidx_t = nc.dram_tensor("gidx", [128, 8 * T_total], mybir.dt.int16,
                            kind="ExternalInput")
    oh_t = nc.dram_tensor("oh", [128, cfg.W * T_total], f16,
                          kind="ExternalInput")
    wT_t = nc.dram_tensor("wT", [cfg.SHARD_PAD, cfg.G], bf16,
                          kind="ExternalInput")
    W1_t = nc.dram_tensor("W1", [cfg.D, cfg.D], bf16, kind="ExternalInput")
    b1_t = nc.dram_tensor("b1", [1, cfg.D], f32, kind="ExternalInput")
    out_t = nc.dram_tensor("partial", [cfg.G, cfg.D], f32,
                           kind="ExternalOutput")

    W, NT, BLK = cfg.W, cfg.NT, cfg.BLK
    add, amax = mybir.AluOpType.add, mybir.AluOpType.max

    with tile.TileContext(nc) as tc:
        with (tc.tile_pool(name="const", bufs=1) as cpool,
              tc.tile_pool(name="gbuf", bufs=8) as gpool,
              tc.tile_pool(name="meta", bufs=8) as mpool,
              tc.tile_pool(name="ohp", bufs=8) as opool,
              tc.tile_pool(name="agg", bufs=2) as apool,
              tc.tile_pool(name="eluv", bufs=2) as epool,
              tc.tile_pool(name="wt", bufs=2) as wpool,
              tc.tile_pool(name="psA", bufs=2, space="PSUM") as pApool,
              tc.tile_pool(name="psB", bufs=2, space="PSUM") as pBpool,
              tc.tile_pool(name="psR", bufs=2, space="PSUM") as pRpool,
              tc.tile_pool(name="psC", bufs=1, space="PSUM") as pCpool,
              tc.tile_pool(name="outp", bufs=1) as outpool):

            # prefetch the first runs' gather metadata before the consts so
            # the first dma_gather can start as early as possible
            pre = {}
            pgoff = 0
            for r in (0, 1):
                T_r = run_T[r]
                gxs = mpool.tile([128, 8 * T_r], mybir.dt.int16)
                nc.sync.dma_start(
                    gxs[:], gidx_t.ap()[:, 8 * pgoff:8 * (pgoff + T_r)])
                ohs = opool.tile([128, T_r, W], f16)
                nc.sync.dma_start(
                    ohs[:].rearrange("p t w -> p (t w)"),
                    oh_t.ap()[:, W * pgoff:W * (pgoff + T_r)])
                pre[r] = (gxs, ohs)
                pgoff += T_r

            ident = cpool.tile([128, 128], f16)
            make_identity(nc, ident[:])
            W1s = cpool.tile([128, cfg.D], bf16)
            nc.sync.dma_start(W1s[:], W1_t.ap())
            b1s = cpool.tile([128, cfg.D], f32)
            nc.sync.dma_start(b1s[:], b1_t.ap().to_broadcast((128, cfg.D)))

            psC = pCpool.tile([cfg.G, cfg.D], f32)

            goff = 0
            for b in range(cfg.N_BLK):
                # init psA with the self-loop term: dinv^2-scaled x rows of
                # this block, host-pretransposed to feat-major.
                xbt = apool.tile([128, BLK], f16, tag="xbt")
                nc.sync.dma_start(
                    xbt[:], xself_t.ap()[:, b * BLK:(b + 1) * BLK])
                psA = pApool.tile([128, BLK], f32)
                nc.tensor.matmul(out=psA[:], lhsT=ident[:], rhs=xbt[:],
                                 start=True, stop=False)
                for ch in range(cfg.N_CHUNK):
                    r = b * cfg.N_CHUNK + ch
                    T_r = run_T[r]
                    S_r = T_r * 128
                    if r in pre:
                        gxs, ohs = pre.pop(r)
                    else:
                        gxs = mpool.tile([128, 8 * T_r], mybir.dt.int16)
                        nc.sync.dma_start(
                            gxs[:], gidx_t.ap()[:, 8 * goff:8 * (goff + T_r)])
                        ohs = opool.tile([128, T_r, W], f16)
                        nc.sync.dma_start(
                            ohs[:].rearrange("p t w -> p (t w)"),
                            oh_t.ap()[:, W * goff:W * (goff + T_r)])

                    gb = gpool.tile([128, T_r, cfg.D], f16)
                    base_row = ch * cfg.CHUNK + cfg.HALF
                    nc.gpsimd.dma_gather(
                        out_ap=gb[:],
                        in_ap=x_t.ap()[base_row:cfg.N, :],
                        idxs_ap=gxs[:],
                        num_idxs=S_r,
                        num_idxs_reg=S_r,
                        elem_size=cfg.D,
                        single_packet=False,
                        queue_num=r % 4,
                    )

                    wbs = run_wb[r]
                    for t in range(T_r):
                        last = (ch == cfg.N_CHUNK - 1 and t == T_r - 1)
                        nc.tensor.matmul(
                            out=psA[:, wbs[t]:wbs[t] + W],
                            lhsT=gb[:, t, :],
                            rhs=ohs[:, t, :],
                            start=False, stop=last,
                        )
                    goff += T_r

                # drain agg (feat-major [D x BLK])
                aggs = apool.tile([128, BLK], bf16)
                nc.vector.tensor_copy(out=aggs[:], in_=psA[:])

                # B: h1 = agg.T @ W1  -> psB [node x feat_out]
                psB = pBpool.tile([128, BLK], f32)
                for nt in range(NT):
                    nc.tensor.matmul(out=psB[:, nt * cfg.D:(nt + 1) * cfg.D],
                                     lhsT=aggs[:, nt * 128:(nt + 1) * 128],
                                     rhs=W1s[:], start=True, stop=True)

                # elu(xb) = relu(xb) - relu(1 - exp(xb)), xb = psB + b1.
                # Every DVE op reads at most ONE SBUF operand (the other is
                # PSUM) -- 2-SBUF-port DVE ops get locked out by concurrent
                # SWDGE descriptor generation (measured 100x slowdown).
                xb = epool.tile([128, NT, cfg.D], f32, tag="xb")
                nc.vector.tensor_tensor(
                    out=xb[:],
                    in0=psB[:].rearrange("p (t d) -> p t d", d=cfg.D),
                    in1=b1s[:].unsqueeze(1).broadcast_to((128, NT, cfg.D)),
                    op=add)
                ex = epool.tile([128, NT * cfg.D], f32, tag="ex")
                nc.scalar.activation(
                    out=ex[:], in_=xb[:].rearrange("p t d -> p (t d)"),
                    func=mybir.ActivationFunctionType.Exp)
                rneg = epool.tile([128, NT * cfg.D], f32, tag="rneg")
                nc.scalar.activation(
                    out=rneg[:], in_=ex[:],
                    func=mybir.ActivationFunctionType.Relu,
                    bias=1.0, scale=-1.0)
                rpos = pRpool.tile([128, NT * cfg.D], f32)
                nc.scalar.activation(
                    out=rpos[:], in_=xb[:].rearrange("p t d -> p (t d)"),
                    func=mybir.ActivationFunctionType.Relu)
                h1e = epool.tile([128, NT * cfg.D], bf16, tag="h1e")
                nc.vector.tensor_tensor(
                    out=h1e[:], in0=rpos[:], in1=rneg[:],
                    op=mybir.AluOpType.subtract)

                # C: partial += wT_block.T @ h1e
                wts = wpool.tile([128, NT, cfg.G], bf16)
                nc.sync.dma_start(
                    wts[:],
                    wT_t.ap()[b * BLK:(b + 1) * BLK, :]
                        .rearrange("(t p) g -> p t g", p=128))
                for nt in range(NT):
                    nc.tensor.matmul(
                        out=psC[:],
                        lhsT=wts[:, nt, :],
                        rhs=h1e[:, nt * cfg.D:(nt + 1) * cfg.D],
                        start=(b == 0 and nt == 0),
                        stop=(b == cfg.N_BLK - 1 and nt == NT - 1),
                    )

            outs = outpool.tile([cfg.G, cfg.D], f32)
            nc.vector.tensor_copy(out=outs[:], in_=psC[:])
            nc.sync.dma_start(out_t.ap(), outs[:])

    nc.compile()
    return nc


# ------------------------------------------------------------- epilogue ----
def epilogue(partials, cnt, W2, b2, fc1_W, fc1_b, fc2_W, fc2_b):
    g_sum = np.sum(partials, axis=0, dtype=np.float32)
    S = g_sum @ W2 + cnt[:, None] * b2[None, :]
    mean = S / np.maximum(cnt, 1.0)[:, None]
    z = np.maximum(mean @ fc1_W + fc1_b[None, :], 0.0)
    z = z @ fc2_W + fc2_b[None, :]
    zmax = z.max(axis=1, keepdims=True)
    lse = np.log(np.sum(np.exp(z - zmax), axis=1, keepdims=True)) + zmax
    return (z - lse).astype(np.float32)


_NC_CACHE = {}


def run_on_device(cfg, per_core, run_T, run_wb, x):
    key = (tuple(run_T), tuple(tuple(w) for w in run_wb))
    if key not in _NC_CACHE:
        _NC_CACHE.clear()
        _NC_CACHE[key] = build_kernel(cfg, run_T, run_wb)
    nc = _NC_CACHE[key]
    xf = np.asarray(x, np.float32)
    xp = np.ascontiguousarray(xf.astype(np.float16))
    in_maps = []
    for c in range(cfg.N_CORES):
        s = per_core[c]
        xs = np.zeros((cfg.SHARD_PAD, cfg.D), dtype=np.float16)
        xs[:cfg.SHARD] = (xf[c * cfg.SHARD:(c + 1) * cfg.SHARD]
                          * s["dinv2"][:, None]).astype(np.float16)
        in_maps.append({
            "x": xp, "x_selfT": np.ascontiguousarray(xs.T),
            "gidx": s["gidx"], "oh": s["oh"],
            "wT": s["wT"], "W1": None, "b1": None,
        })
    return nc, in_maps


def kernel(x, edge_index, batch, W1, b1, W2, b2, fc1_W, fc1_b, fc2_W, fc2_b):
    from concourse import bass_utils
    cfg = FULL
    per_core, cnt, run_T, run_wb = preprocess(cfg, edge_index, batch)
    nc, in_maps = run_on_device(cfg, per_core, run_T, run_wb, x)
    import ml_dtypes
    W1f = np.ascontiguousarray(
        np.asarray(W1, dtype=np.float32).astype(ml_dtypes.bfloat16))
    b1f = np.asarray(b1, dtype=np.float32).reshape(1, cfg.D)
    for m in in_maps:
        m["W1"] = W1f
        m["b1"] = b1f
    res = bass_utils.run_bass_kernel_spmd(
        nc, in_maps, core_ids=list(range(cfg.N_CORES)))
    partials = [res.results[c]["partial"] for c in range(cfg.N_CORES)]
    out = epilogue(partials, cnt,
                   np.asarray(W2, np.float32), np.asarray(b2, np.float32),
                   np.asarray(fc1_W, np.float32), np.asarray(fc1_b, np.float32),
                   np.asarray(fc2_W, np.float32), np.asarray(fc2_b, np.float32))
    return out
